# revision 37
# baseline (speedup 1.0000x reference)
import sys
sys.path.insert(0, '/opt/trn_rl_repo')
import contextlib
import numpy as np
import ml_dtypes

import concourse.bass as bass
import concourse.mybir as mybir
import concourse.tile as tile
from concourse import bacc
from concourse.bass_utils import run_bass_kernel_spmd

f32 = mybir.dt.float32
bf16 = mybir.dt.bfloat16
AF = mybir.ActivationFunctionType
OP = mybir.AluOpType
X = mybir.AxisListType.X
NCORES = 8
C = 64
R = 8
CR = C // R
bf = ml_dtypes.bfloat16


def _to_bf(a):
    return np.ascontiguousarray(np.asarray(a, np.float32)).astype(bf)


def _f32(a, shape=None):
    a = np.ascontiguousarray(np.asarray(a, np.float32))
    return a.reshape(shape) if shape is not None else a


def prep_conv_lhsT(w):
    out = np.zeros((6, 128, 64), np.float32)
    for g in range(3):
        dy = g - 1
        out[g, 0:64] = w[:, :, dy + 1, 0].T   # tap (dy,-1) at anchor
        out[g, 64:128] = w[:, :, dy + 1, 1].T  # tap (dy,0) via +1-shifted dup
    for g in range(3, 6):
        dy = g - 4
        out[g, 0:64] = w[:, :, dy + 1, 2].T   # tap (dy,+1), K=64
    return _to_bf(out)


def prep_params(params):
    p = params
    d = {}
    d['c1_lhsT'] = prep_conv_lhsT(_f32(p['conv1_w']))
    d['c2_lhsT'] = prep_conv_lhsT(_f32(p['conv2_w']))
    d['c1_bias'] = np.tile(_f32(p['conv1_b']), 2).reshape(128, 1).astype(np.float32)
    idp = np.zeros((128, 64), np.float32)
    idp[0:64] = -np.eye(64)
    idp[64:128] = np.eye(64)
    d['idpair'] = _to_bf(idp)

    for i, tdp in enumerate([p['tdp1'], p['tdp2']], start=1):
        cv_w = _f32(tdp['cv_w'])
        c1w, c2w = _f32(tdp['c1_w']), _f32(tdp['c2_w'])
        c1b, c2b = _f32(tdp['c1_b']), _f32(tdp['c2_b'])
        cv0w, cv1w = _f32(tdp['cv0_w']).reshape(CR), _f32(tdp['cv1_w']).reshape(CR)
        cv0b = float(_f32(tdp['cv0_b']).reshape(()))
        cv1b = float(_f32(tdp['cv1_b']).reshape(()))
        S1, S2 = c1w.sum(1), c2w.sum(1)
        t2b = np.zeros((128, 32), np.float32)
        t2b[0:64, 0:CR] = cv_w.T
        t2b[64:128, 0:CR] = cv_w.T
        d[f't2base{i}'] = _to_bf(t2b)  # [128,32], cols 8:32 zero
        cb = np.zeros((128, 1), np.float32)
        for g in range(4):
            cb[32 * g:32 * g + CR, 0] = _f32(tdp['cv_b'])
        d[f'cvb4_{i}'] = cb
        M = np.outer(cv0w, S1) - np.outer(cv1w, S2)
        Mrep = np.zeros((128, 64), np.float32)
        for g in range(4):
            Mrep[32 * g:32 * g + CR] = M
        d[f'M{i}'] = _to_bf(Mrep)
        d[f'dconst{i}'] = ((c1b - c2b) + S1 * cv0b - S2 * cv1b).reshape(64, 1).astype(np.float32)
        d[f'fcT{i}'] = _to_bf(_f32(tdp['fc_w']).T)
        d[f'fcb{i}'] = _f32(tdp['fc_b'], (CR, 1))
        d[f'fc01T{i}'] = _to_bf(np.concatenate([_f32(tdp['fc0_w']).T, _f32(tdp['fc1_w']).T], axis=1))
        d[f'fc01b{i}'] = np.concatenate([_f32(tdp['fc0_b']), _f32(tdp['fc1_b'])]).reshape(128, 1).astype(np.float32)
        d[f'dmatT{i}'] = _to_bf(np.concatenate([c1w.T, -c2w.T], axis=0))

    d['caw1T'] = _to_bf(_f32(p['ca_w1']).T)
    d['cab1'] = _f32(p['ca_b1'], (CR, 1))
    d['caw2T'] = _to_bf(_f32(p['ca_w2']).T)
    d['cab2'] = _f32(p['ca_b2'], (64, 1))
    paw1T = _f32(p['pa_w1']).T
    pa1t = np.zeros((128, 32), np.float32)
    pa1t[64:128, 0:CR] = paw1T
    d['pa1T'] = _to_bf(pa1t)
    pb = np.zeros((128, 1), np.float32)
    for g in range(4):
        pb[32 * g:32 * g + CR, 0] = _f32(p['pa_b1'])
    d['pa1b4'] = pb
    d['pab1v'] = _f32(p['pa_b1'], (CR, 1))
    paw2 = _f32(p['pa_w2']).reshape(CR)
    p2r = np.zeros((128, 64), np.float32)
    for g in range(4):
        p2r[32 * g:32 * g + CR] = np.repeat(paw2.reshape(CR, 1), 64, axis=1)
    d['pa2rep'] = _to_bf(p2r)
    c2bv = _f32(p['conv2_b'], (64, 1))
    d['c2b128'] = np.tile(c2bv, (2, 1)).astype(np.float32)          # [128,1] f32
    d['c2b128b'] = _to_bf(np.tile(c2bv, (2, 1)))                    # [128,1] bf16
    return d


SHAPES_BF = {'c1_lhsT': [6, 128, 64], 'c2_lhsT': [6, 128, 64], 'idpair': [128, 64],
             't2base1': [128, 32], 't2base2': [128, 32], 'M1': [128, 64], 'M2': [128, 64],
             'fcT1': [64, 8], 'fcT2': [64, 8], 'fc01T1': [8, 128], 'fc01T2': [8, 128],
             'dmatT1': [128, 64], 'dmatT2': [128, 64], 'caw1T': [64, 8], 'caw2T': [8, 64],
             'pa1T': [128, 32], 'pa2rep': [128, 64], 'c2b128b': [128, 1]}
SHAPES_F32 = {'c1_bias': [128, 1], 'cvb4_1': [128, 1], 'cvb4_2': [128, 1],
              'dconst1': [64, 1], 'dconst2': [64, 1], 'fcb1': [CR, 1], 'fcb2': [CR, 1],
              'fc01b1': [128, 1], 'fc01b2': [128, 1], 'cab1': [CR, 1], 'cab2': [64, 1],
              'pa1b4': [128, 1], 'pab1v': [CR, 1], 'c2b128': [128, 1]}


def build_program(H_sh, W, sc, use_cc=True, phases='ABCD'):
    """sc: baked floats: twf1 (2x2 list), twf1b (2), twf2, twf2b, pa2b, inv_hw, inv_c."""
    Wp = W + 2
    nbands = H_sh // 16
    assert H_sh % 16 == 0
    NP2 = 2 * W                   # positions per 2-row chunk
    L = (H_sh + 4) * Wp
    HWf = 1.0 / sc['inv_hw']      # full image H*W

    nc = bacc.Bacc("TRN2", target_bir_lowering=False, debug=False, num_devices=NCORES)

    XS = nc.dram_tensor("xs", [64, L], bf16, kind="ExternalInput")
    DIN = {}
    for n, s in SHAPES_BF.items():
        DIN[n] = nc.dram_tensor(n, s, bf16, kind="ExternalInput")
    for n, s in SHAPES_F32.items():
        DIN[n] = nc.dram_tensor(n, s, f32, kind="ExternalInput")
    MYB = nc.dram_tensor("myb", [64, 4], f32, kind="ExternalInput")
    EDGE = nc.dram_tensor("edgemask", [64, 2], f32, kind="ExternalInput")
    OUT = nc.dram_tensor("out", [64, H_sh * W], f32, kind="ExternalOutput")

    with tile.TileContext(nc) as tc, contextlib.ExitStack() as ctx:
        sbc = ctx.enter_context(tc.tile_pool(name="consts", bufs=1))
        sbig = ctx.enter_context(tc.tile_pool(name="big", bufs=1))
        sdup = ctx.enter_context(tc.tile_pool(name="dup", bufs=3))
        swork = ctx.enter_context(tc.tile_pool(name="work", bufs=3))
        sst = ctx.enter_context(tc.tile_pool(name="stats", bufs=1))
        dram = ctx.enter_context(tc.tile_pool(name="drambb", bufs=1, space="DRAM"))
        cpsP = ctx.enter_context(tc.tile_pool(name="cpsP", bufs=2, space="PSUM"))
        t2psP = ctx.enter_context(tc.tile_pool(name="t2psP", bufs=2, space="PSUM"))
        dpsP = ctx.enter_context(tc.tile_pool(name="dpsP", bufs=2, space="PSUM"))

        SB = {}
        for n, s in SHAPES_BF.items():
            if n in ('c1_lhsT', 'c2_lhsT'):
                groups = []
                for g in range(6):
                    t = sbc.tile([128, 64], bf16, tag=f"{n}_{g}")
                    nc.sync.dma_start(out=t[:, :], in_=DIN[n].ap()[g, :, :])
                    groups.append(t)
                SB[n] = groups
                continue
            t = sbc.tile(s, bf16, tag=n)
            nc.sync.dma_start(out=t[(slice(None),) * len(s)], in_=DIN[n].ap())
            SB[n] = t
        for n, s in SHAPES_F32.items():
            t = sbc.tile(s, f32, tag=n)
            nc.sync.dma_start(out=t[:, :], in_=DIN[n].ap())
            SB[n] = t
        myb = sbc.tile([64, 4], f32, tag="myb")
        nc.sync.dma_start(out=myb[:, :], in_=MYB.ap())
        edgem = sbc.tile([64, 2], f32, tag="edgem")
        nc.sync.dma_start(out=edgem[:, :], in_=EDGE.ap())

        XR = sbig.tile([128, L], bf16, tag="XR")
        xr = XR[:, :]
        PSTR = xr.ap[0][0]
        xs_ap = XS.ap()

        for i in range(4):
            s0 = i * (L // 4)
            s1 = (i + 1) * (L // 4) if i < 3 else L
            nc.sync.dma_start(out=XR[0:64, s0:s1], in_=xs_ap[:, s0:s1])

        def xrap(half, r, nrows, w0=1, wn=None):
            """[64, nrows, wn] view at out-rows r.. (flat (r+2)*Wp + w0)."""
            wn = W if wn is None else wn
            return bass.AP(tensor=XR.tensor,
                           offset=xr.offset + (64 * PSTR if half else 0) + (r + 2) * Wp + w0,
                           ap=[[PSTR, 64], [Wp, nrows], [1, wn]])

        def xr128(r, nrows):
            """[128, nrows, W] both halves."""
            return bass.AP(tensor=XR.tensor, offset=xr.offset + (r + 2) * Wp + 1,
                           ap=[[PSTR, 128], [Wp, nrows], [1, W]])

        def dbg_dump_res():
            for rr in range(0, H_sh, 4):
                ot = swork.tile([128, 4 * W], f32, tag="outst")
                nc.scalar.activation(out=ot[0:64, 0:4 * W], in_=xrap(1, rr, 4), func=AF.Copy)
                nc.sync.dma_start(out=OUT.ap()[:, rr * W:(rr + 4) * W],
                                  in_=ot[0:64, 0:4 * W])

        # ---- stats tiles
        st_res = sst.tile([64, 72], f32, tag="st_res")
        st_x = sst.tile([64, 16], f32, tag="st_x")
        st_r2 = sst.tile([64, 72], f32, tag="st_r2")
        st_r3 = sst.tile([64, 40], f32, tag="st_r3")
        zero64 = sst.tile([64, 1], f32, tag="zero64")
        nc.vector.memset(zero64[:, :], 0.0)

        # =========================================================
        # generic conv (used for conv1 in A and conv2 in B)
        def conv_emit(cpool, lhsT6, dup_t, dup_r0, r0, bias128, func, stats_t, scol):
            """8 output rows r0..r0+8 -> two [128, NP2] psum tiles (4 rows each)."""
            dap = dup_t[:, :]
            dstr = dap.ap[0][0]
            for sl in range(2):
                ps = cpool.tile([128, NP2], f32, tag="cps")
                for cp in range(2):
                    rp = r0 + sl * 4 + 2 * cp
                    for g in range(6):
                        dy = (g % 3) - 1
                        anchor = (rp + dy - dup_r0) * Wp + (0 if g < 3 else 2)
                        if g < 3:
                            lw, npart = lhsT6[g][:, :], 128
                        else:
                            lw, npart = lhsT6[g][0:64, :], 64
                        rhs = bass.AP(tensor=dup_t.tensor, offset=dap.offset + anchor,
                                      ap=[[dstr, npart], [Wp, 2], [1, W]])
                        nc.tensor.matmul(out=ps[64 * cp:64 * cp + 64, :],
                                         lhsT=lw, rhs=rhs, start=(g == 0), stop=(g == 5),
                                         tile_position=(0, 64 * cp))
                for cp in range(2):
                    rbase = r0 + sl * 4 + 2 * cp
                    dst = bass.AP(tensor=XR.tensor,
                                  offset=xr.offset + 64 * PSTR + (rbase + 2) * Wp + 1,
                                  ap=[[PSTR, 64], [Wp, 2], [1, W]])
                    src = bass.AP(tensor=ps.tensor,
                                  offset=ps[:, :].offset + (64 * cp) * ps[:, :].ap[0][0],
                                  ap=[[ps[:, :].ap[0][0], 64], [1, NP2]])
                    nc.scalar.activation(out=dst, in_=src, func=func,
                                         bias=bias128[64 * cp:64 * cp + 64, :] if func != AF.Copy else 0.0,
                                         accum_out=stats_t[:, scol[0]:scol[0] + 1])
                    scol[0] += 1

        # ================= PHASE A : conv1 =================
        if True:
            cpsA = cpsP
            scol = [0]
            for b_ in range(nbands):
                r0 = 16 * b_
                er0 = r0 - 2 if b_ == 0 else r0 - 1
                er1 = r0 + 18 if b_ == nbands - 1 else r0 + 17
                nr = er1 - er0
                dup = sdup.tile([128, 21 * Wp], bf16, tag="dupT")
                flen = nr * Wp
                nc.sync.dma_start(
                    out=dup[0:64, 0:flen],
                    in_=bass.AP(tensor=xs_ap.tensor, offset=xs_ap.offset + (er0 + 2) * Wp,
                                ap=[[xs_ap.ap[0][0], 64], [1, flen]]))
                nc.sync.dma_start(
                    out=dup[64:128, 0:flen - 1],
                    in_=bass.AP(tensor=xs_ap.tensor, offset=xs_ap.offset + (er0 + 2) * Wp + 1,
                                ap=[[xs_ap.ap[0][0], 64], [1, flen - 1]]))
                nc.vector.memset(dup[64:128, flen - 1:flen], 0.0)
                for g8 in range(2):
                    conv_emit(cpsA, SB['c1_lhsT'], dup, er0, r0 + 8 * g8,
                              SB['c1_bias'], AF.Relu, st_res, scol)
                if b_ == 0 or b_ == nbands - 1:
                    rr = -1 if b_ == 0 else H_sh
                    eps = cpsA.tile([128, NP2], f32, tag="cps")
                    for g in range(6):
                        dy = (g % 3) - 1
                        anchor = (rr + dy - er0) * Wp + (0 if g < 3 else 2)
                        if g < 3:
                            lw, npart = SB['c1_lhsT'][g][:, :], 128
                        else:
                            lw, npart = SB['c1_lhsT'][g][0:64, :], 64
                        rhs = bass.AP(tensor=dup.tensor, offset=dup[:, :].offset + anchor,
                                      ap=[[dup[:, :].ap[0][0], npart], [Wp, 1], [1, W]])
                        nc.tensor.matmul(out=eps[0:64, 0:W], lhsT=lw, rhs=rhs,
                                         start=(g == 0), stop=(g == 5), tile_position=(0, 0))
                    nc.scalar.activation(out=xrap(1, rr, 1), in_=eps[0:64, 0:W],
                                         func=AF.Relu, bias=SB['c1_bias'][0:64, :])
            for b_ in range(nbands):
                nc.vector.tensor_reduce(
                    out=st_x[:, b_:b_ + 1],
                    in_=bass.AP(tensor=XR.tensor, offset=xr.offset + (16 * b_ + 2) * Wp,
                                ap=[[PSTR, 64], [1, 16 * Wp]]),
                    axis=X, op=OP.add)
            nconv_cols = scol[0]

        res_sum = sst.tile([64, 1], f32, tag="res_sum")
        x_sum = sst.tile([64, 1], f32, tag="x_sum")
        nc.vector.tensor_reduce(out=res_sum[:, :], in_=st_res[:, 0:nconv_cols], axis=X, op=OP.add)
        nc.vector.tensor_reduce(out=x_sum[:, :], in_=st_x[:, 0:nbands], axis=X, op=OP.add)

        if phases == 'A':
            dbg_dump_res()
        # ---- AllReduce #1: [8, 64] rows = 2*b + kind
        ar1sb = sst.tile([64, 8], f32, tag="ar1sb")
        for bb in range(4):
            nc.vector.tensor_tensor(out=ar1sb[:, 2 * bb:2 * bb + 1], in0=res_sum[:, :],
                                    in1=myb[:, bb:bb + 1], op=OP.mult)
            nc.vector.tensor_tensor(out=ar1sb[:, 2 * bb + 1:2 * bb + 2], in0=x_sum[:, :],
                                    in1=myb[:, bb:bb + 1], op=OP.mult)
        ar1i = dram.tile([8, 64], f32, tag="ar1i")
        ar1o = dram.tile([8, 64], f32, tag="ar1o")
        for k in range(8):
            nc.sync.dma_start(out=ar1i[k:k + 1, :], in_=ar1sb[:, k:k + 1])
        if use_cc:
            nc.gpsimd.collective_compute("AllReduce", OP.add, replica_groups=[list(range(NCORES))],
                                         ins=[ar1i.opt()], outs=[ar1o.opt()])
        else:
            nc.sync.dma_start(out=ar1o[:, :], in_=ar1i[:, :])
        arg1 = sst.tile([64, 8], f32, tag="arg1")   # global sums, col k = row k
        for k in range(8):
            nc.sync.dma_start(out=arg1[:, k:k + 1], in_=ar1o[k:k + 1, :])


        # =========================================================
        # small helpers for phase preambles
        wbb = dram.tile([1, 4], f32, tag="wbb")   # scalar broadcast bounce

        def sel_my(dst, src_tile, cols):
            """dst[64,1] = sum_b src[:, cols[b]] * myb[:, b]."""
            nc.vector.tensor_tensor(out=dst, in0=src_tile[:, cols[0]:cols[0] + 1],
                                    in1=myb[:, 0:1], op=OP.mult)
            for b2 in range(1, 4):
                nc.vector.scalar_tensor_tensor(out=dst, in0=src_tile[:, cols[b2]:cols[b2] + 1],
                                               scalar=myb[:, b2:b2 + 1], in1=dst,
                                               op0=OP.mult, op1=OP.add)

        def tdp_scalar_math(i, pool, rsum_my, xsum_my, rs0, xs0, twf, twfb):
            """Compute wvec128(scaled t2 lhsT), w1v0/w2v0, Dp (sigmoid bias) for tdp i.

            rsum_my/xsum_my: [64,1] global sums for my sample; rs0/xs0: [64,1] for b=0.
            Returns (t2ws, Dp)."""
            # two_weight: m = inv_chw * sum_c(sums0) ; tw = relu(twf@m + twfb)
            mr = sst.tile([1, 2], f32, tag=f"mr{i}")
            nc.gpsimd.tensor_reduce(out=mr[0:1, 0:1], in_=rs0, axis=mybir.AxisListType.C, op=OP.add)
            nc.gpsimd.tensor_reduce(out=mr[0:1, 1:2], in_=xs0, axis=mybir.AxisListType.C, op=OP.add)
            inv_chw = sc['inv_hw'] * sc['inv_c']
            tw = sst.tile([1, 2], f32, tag=f"tw{i}")
            twbt = sst.tile([1, 2], f32, tag=f"twbt{i}")
            for j in range(2):
                nc.vector.memset(twbt[0:1, j:j + 1], float(twfb[j]))
                nc.vector.tensor_scalar(out=tw[0:1, j:j + 1], in0=mr[0:1, 0:1],
                                        scalar1=twf[j][0] * inv_chw, scalar2=None, op0=OP.mult)
                nc.vector.scalar_tensor_tensor(out=tw[0:1, j:j + 1], in0=mr[0:1, 1:2],
                                               scalar=twf[j][1] * inv_chw, in1=tw[0:1, j:j + 1],
                                               op0=OP.mult, op1=OP.add)
                nc.scalar.activation(out=tw[0:1, j:j + 1], in_=tw[0:1, j:j + 1],
                                     func=AF.Relu, bias=twbt[0:1, j:j + 1])
            # broadcast w1 (res scale), w2 (x scale)
            nc.sync.dma_start(out=wbb[0:1, 0:2], in_=tw[0:1, 0:2])
            wvec = sst.tile([128, 1], f32, tag=f"wvec{i}")
            w1v0 = sst.tile([64, 2], f32, tag=f"w1v0{i}")
            nc.sync.dma_start(out=wvec[0:64, 0:1],
                              in_=bass.AP(tensor=wbb.tensor, offset=wbb[:, :].offset + 1,
                                          ap=[[0, 64], [1, 1]]))
            nc.sync.dma_start(out=wvec[64:128, 0:1],
                              in_=bass.AP(tensor=wbb.tensor, offset=wbb[:, :].offset + 0,
                                          ap=[[0, 64], [1, 1]]))
            nc.sync.dma_start(out=w1v0[0:64, 0:1],
                              in_=bass.AP(tensor=wbb.tensor, offset=wbb[:, :].offset + 0,
                                          ap=[[0, 64], [1, 1]]))
            nc.sync.dma_start(out=w1v0[0:64, 1:2],
                              in_=bass.AP(tensor=wbb.tensor, offset=wbb[:, :].offset + 1,
                                          ap=[[0, 64], [1, 1]]))
            t2ws = sst.tile([128, 32], bf16, tag=f"t2ws{i}")
            nc.vector.tensor_scalar(out=t2ws[:, :], in0=SB[f't2base{i}'][:, :],
                                    scalar1=wvec[:, 0:1], scalar2=None, op0=OP.mult)
            # temp = inv_hw * (w1*rsum_my + w2*xsum_my)
            ta = sst.tile([64, 2], f32, tag=f"ta{i}")
            nc.vector.tensor_scalar(out=ta[:, 0:1], in0=xsum_my, scalar1=w1v0[:, 1:2],
                                    scalar2=sc['inv_hw'], op0=OP.mult, op1=OP.mult)
            nc.vector.tensor_scalar(out=ta[:, 1:2], in0=rsum_my, scalar1=w1v0[:, 0:1],
                                    scalar2=sc['inv_hw'], op0=OP.mult, op1=OP.mult)
            tempb = sst.tile([64, 1], bf16, tag=f"tempb{i}")
            nc.vector.tensor_tensor(out=tempb[:, :], in0=ta[:, 0:1], in1=ta[:, 1:2], op=OP.add)
            # t1 = relu(fcT.T @ temp + fcb)
            tps = pool.tile([128, 2 * NP2], f32, tag="dps")
            nc.tensor.matmul(out=tps[0:CR, 0:1], lhsT=SB[f'fcT{i}'][:, :], rhs=tempb[:, :],
                             start=True, stop=True, tile_position=(0, 0))
            t1b = sst.tile([CR, 1], bf16, tag=f"t1b{i}")
            nc.scalar.activation(out=t1b[:, :], in_=tps[0:CR, 0:1], func=AF.Relu,
                                 bias=SB[f'fcb{i}'][:, :])
            # ab = fc01T.T @ t1 + fc01b ; D = dmatT.T @ ab ; Dp = D + dconst (to hi half)
            nc.tensor.matmul(out=tps[0:128, 2:3], lhsT=SB[f'fc01T{i}'][:, :], rhs=t1b[:, :],
                             start=True, stop=True, tile_position=(0, 0))
            absb = sst.tile([128, 1], bf16, tag=f"absb{i}")
            nc.vector.tensor_scalar(out=absb[:, :], in0=tps[0:128, 2:3],
                                    scalar1=SB[f'fc01b{i}'][:, :], scalar2=None, op0=OP.add)
            nc.tensor.matmul(out=tps[0:64, 4:5], lhsT=SB[f'dmatT{i}'][:, :], rhs=absb[:, :],
                             start=True, stop=True, tile_position=(0, 0))
            Dp0 = sst.tile([64, 1], f32, tag=f"dp0{i}")
            nc.vector.tensor_scalar(out=Dp0[:, :], in0=tps[0:64, 4:5],
                                    scalar1=SB[f'dconst{i}'][:, :], scalar2=None, op0=OP.add)
            Dp = sst.tile([128, 1], f32, tag=f"dp{i}")
            nc.scalar.activation(out=Dp[64:128, :], in_=Dp0[:, :], func=AF.Copy)
            return t2ws, Dp

        def tdp_band(i, r0, nrows, t2ws, Dp, t2pool, dpool, out_cb):
            """Emit tdp attention for rows [r0, r0+nrows). out_cb(pair_idx, rfirst,
            nprows, v_psum_ap) consumes v (=s*(res-x)) chunks (4 rows per call,
            or fewer at the tail)."""
            chunks = nrows // 2
            ci = 0
            pair = 0
            while ci < chunks:
                take = min(4, chunks - ci)       # chunks in this t2 psum tile
                t2ps = t2pool.tile([128, NP2], f32, tag="t2ps")
                for k in range(take):
                    rr = r0 + 2 * (ci + k)
                    nc.tensor.matmul(out=t2ps[32 * k:32 * k + 32, :], lhsT=t2ws[:, :],
                                     rhs=xr128(rr, 2), start=True, stop=True,
                                     tile_position=(0, 32 * k))
                t2sb = swork.tile([128, NP2], bf16, tag="t2sb")
                tp = 32 * take
                nc.scalar.activation(out=t2sb[0:tp, :], in_=t2ps[0:tp, :], func=AF.Relu,
                                     bias=SB[f'cvb4_{i}'][0:tp, :])
                # pairs of chunks -> dups tiles
                k = 0
                while k < take:
                    tk = min(2, take - k)
                    dps = dpool.tile([128, 2 * NP2], f32, tag="dps")
                    for q in range(tk):
                        g = k + q
                        rr = r0 + 2 * (ci + g)
                        nc.tensor.matmul(out=dps[64:128, q * NP2:(q + 1) * NP2],
                                         lhsT=SB[f'M{i}'][32 * g:32 * g + CR, :],
                                         rhs=t2sb[32 * g:32 * g + CR, :],
                                         start=True, stop=True, tile_position=(32 * g, 64))
                        nc.tensor.matmul(out=dps[0:64, q * NP2:(q + 1) * NP2],
                                         lhsT=SB['idpair'][:, :], rhs=xr128(rr, 2),
                                         start=True, stop=True, tile_position=(0, 0))
                    fl = tk * NP2
                    s_sb = swork.tile([128, 2 * NP2], bf16, tag="s_sb")
                    nc.scalar.activation(out=s_sb[64:128, 0:fl], in_=dps[64:128, 0:fl],
                                         func=AF.Sigmoid, bias=Dp[64:128, :])
                    nc.vector.tensor_tensor(out=dps[0:64, 0:fl], in0=s_sb[64:128, 0:fl],
                                            in1=dps[0:64, 0:fl], op=OP.mult)
                    out_cb(pair, r0 + 2 * (ci + k), 2 * tk, dps)
                    pair += 1
                    k += tk
                ci += take

        # ================= PHASE B : tdp1 + conv2 =================
        if phases != 'A':
            t2psB, dupsB, cpsB = t2psP, dpsP, cpsP
            rs_my = sst.tile([64, 1], f32, tag="rs_my")
            xs_my = sst.tile([64, 1], f32, tag="xs_my")
            sel_my(rs_my[:, :], arg1, [0, 2, 4, 6])
            sel_my(xs_my[:, :], arg1, [1, 3, 5, 7])
            t2ws1, Dp1 = tdp_scalar_math(1, dupsB, rs_my[:, :], xs_my[:, :],
                                         arg1[:, 0:1], arg1[:, 1:2],
                                         sc['twf1'], sc['twf1b'])
            scol2 = [0]
            dup_tiles = {}

            def make_band(b_):
                r0 = 16 * b_
                c2dup = sdup.tile([128, 21 * Wp], bf16, tag="dupT")
                dr0 = r0 - 1
                nr = 18
                dstr = c2dup[:, :].ap[0][0]
                nc.vector.memset(bass.AP(tensor=c2dup.tensor, offset=c2dup[:, :].offset,
                                         ap=[[dstr, 64], [Wp, nr], [1, 1]]), 0.0)
                nc.vector.memset(bass.AP(tensor=c2dup.tensor, offset=c2dup[:, :].offset + Wp - 1,
                                         ap=[[dstr, 64], [Wp, nr], [1, 1]]), 0.0)

                def res1_cb(pair, rfirst, nprows, dps, _dup=c2dup, _dr0=dr0):
                    dst = bass.AP(tensor=_dup.tensor,
                                  offset=_dup[:, :].offset + (rfirst - _dr0) * Wp + 1,
                                  ap=[[_dup[:, :].ap[0][0], 64], [Wp, nprows], [1, W]])
                    nc.vector.tensor_tensor(out=dst, in0=xrap(0, rfirst, nprows),
                                            in1=dps[0:64, 0:nprows * W], op=OP.add)

                tdp_band(1, r0 - 1, 18, t2ws1, Dp1, t2psB, dupsB, res1_cb)
                # zero res1 halo rows at the global image edges
                if b_ == 0:
                    nc.vector.tensor_scalar(out=c2dup[0:64, 0:Wp], in0=c2dup[0:64, 0:Wp],
                                            scalar1=edgem[:, 0:1], scalar2=None, op0=OP.mult)
                if b_ == nbands - 1:
                    nc.vector.tensor_scalar(out=c2dup[0:64, 17 * Wp:18 * Wp],
                                            in0=c2dup[0:64, 17 * Wp:18 * Wp],
                                            scalar1=edgem[:, 1:2], scalar2=None, op0=OP.mult)
                flen = nr * Wp
                nc.sync.dma_start(out=c2dup[64:128, 0:flen - 1],
                                  in_=bass.AP(tensor=c2dup.tensor,
                                              offset=c2dup[:, :].offset + 1,
                                              ap=[[dstr, 64], [1, flen - 1]]))
                nc.vector.memset(c2dup[64:128, flen - 1:flen], 0.0)
                dup_tiles[b_] = (c2dup, dr0)

            def conv2_band(b_):
                c2dup, dr0 = dup_tiles.pop(b_)
                for g8 in range(2):
                    conv_emit(cpsB, SB['c2_lhsT'], c2dup, dr0, 16 * b_ + 8 * g8,
                              None, AF.Copy, st_r2, scol2)

            make_band(0)
            for b_ in range(1, nbands):
                make_band(b_)
                conv2_band(b_ - 1)
            conv2_band(nbands - 1)
            r2_sum = sst.tile([64, 1], f32, tag="r2_sum")
            nc.vector.tensor_reduce(out=r2_sum[:, :], in_=st_r2[:, 0:scol2[0]], axis=X, op=OP.add)

        if phases == 'AB':
            dbg_dump_res()
        # ---- AllReduce #2: [4, 64] res2 sums (bias-less)
        ar2sb = sst.tile([64, 4], f32, tag="ar2sb")
        for bb in range(4):
            nc.vector.tensor_tensor(out=ar2sb[:, bb:bb + 1], in0=r2_sum[:, :],
                                    in1=myb[:, bb:bb + 1], op=OP.mult)
        ar2i = dram.tile([4, 64], f32, tag="ar2i")
        ar2o = dram.tile([4, 64], f32, tag="ar2o")
        for k in range(4):
            nc.sync.dma_start(out=ar2i[k:k + 1, :], in_=ar2sb[:, k:k + 1])
        if use_cc:
            nc.gpsimd.collective_compute("AllReduce", OP.add, replica_groups=[list(range(NCORES))],
                                         ins=[ar2i.opt()], outs=[ar2o.opt()])
        else:
            nc.sync.dma_start(out=ar2o[:, :], in_=ar2i[:, :])
        arg2 = sst.tile([64, 4], f32, tag="arg2")
        for k in range(4):
            nc.sync.dma_start(out=arg2[:, k:k + 1], in_=ar2o[k:k + 1, :])

        # ================= PHASE C : CA + PA =================
        if phases.startswith('ABC') and phases != 'ABCD2':
            papsC, dupsC = t2psP, dpsP
            r2my = sst.tile([64, 1], f32, tag="r2my")
            sel_my(r2my[:, :], arg2, [0, 1, 2, 3])
            # true sums = stored + conv2_b * HW ; m_ca = inv_hw * true
            mca = sst.tile([64, 1], bf16, tag="mca")
            mca32 = sst.tile([64, 1], f32, tag="mca32")
            nc.vector.scalar_tensor_tensor(out=mca32[:, :], in0=SB['c2b128'][0:64, :],
                                           scalar=HWf, in1=r2my[:, :], op0=OP.mult, op1=OP.add)
            nc.vector.tensor_scalar(out=mca[:, :], in0=mca32[:, :], scalar1=sc['inv_hw'],
                                    scalar2=None, op0=OP.mult)
            cps = dupsC.tile([128, 2 * NP2], f32, tag="dps")
            nc.tensor.matmul(out=cps[0:CR, 0:1], lhsT=SB['caw1T'][:, :], rhs=mca[:, :],
                             start=True, stop=True, tile_position=(0, 0))
            tca = sst.tile([CR, 1], bf16, tag="tca")
            nc.scalar.activation(out=tca[:, :], in_=cps[0:CR, 0:1], func=AF.Relu,
                                 bias=SB['cab1'][:, :])
            nc.tensor.matmul(out=cps[0:64, 2:3], lhsT=SB['caw2T'][:, :], rhs=tca[:, :],
                             start=True, stop=True, tile_position=(0, 0))
            ca128 = sst.tile([128, 1], f32, tag="ca128")
            nc.vector.memset(ca128[:, :], 0.0)
            nc.scalar.activation(out=ca128[64:128, :], in_=cps[0:64, 2:3], func=AF.Sigmoid,
                                 bias=SB['cab2'][:, :])
            cab = sst.tile([128, 1], f32, tag="cab")
            nc.vector.tensor_tensor(out=cab[64:128, :], in0=ca128[64:128, :],
                                    in1=SB['c2b128'][64:128, :], op=OP.mult)
            pa2bt = sst.tile([128, 1], f32, tag="pa2bt")
            nc.vector.memset(pa2bt[:, :], float(sc['pa2b']))
            pa1s = sst.tile([128, 32], bf16, tag="pa1s")
            nc.vector.tensor_scalar(out=pa1s[:, :], in0=SB['pa1T'][:, :],
                                    scalar1=ca128[:, 0:1], scalar2=None, op0=OP.mult)
            # pa1 effective bias: pa_b1 + pa1s.T @ c2b
            nc.tensor.matmul(out=cps[0:32, 4:5], lhsT=pa1s[:, :],
                             rhs=SB['c2b128b'][:, :], start=True, stop=True,
                             tile_position=(0, 0))
            pabe = sst.tile([128, 1], f32, tag="pabe")
            nc.vector.memset(pabe[:, :], 0.0)
            nc.vector.tensor_scalar(out=pabe[0:CR, 0:1], in0=cps[0:CR, 4:5],
                                    scalar1=SB['pab1v'][:, :], scalar2=None, op0=OP.add)
            for g in range(1, 4):
                nc.scalar.activation(out=pabe[32 * g:32 * g + CR, :], in_=pabe[0:CR, 0:1],
                                     func=AF.Copy)
            csub = int(phases[3]) if len(phases) > 3 and phases[3].isdigit() else 9
            st_r3b = sst.tile([128, 40], f32, tag="st_r3b")
            scol3 = [0]
            for b_ in range(nbands) if csub >= 2 else []:
                r0 = 16 * b_
                for half8 in range(2):
                    rh = r0 + 8 * half8
                    paps = papsC.tile([128, NP2], f32, tag="t2ps")
                    for k in range(4):
                        rr = rh + 2 * k
                        nc.tensor.matmul(out=paps[32 * k:32 * k + 32, :],
                                         lhsT=pa1s[:, :], rhs=xr128(rr, 2),
                                         start=True, stop=True, tile_position=(0, 32 * k))
                    pat = swork.tile([128, NP2], bf16, tag="t2sb")
                    nc.scalar.activation(out=pat[:, :], in_=paps[:, :], func=AF.Relu,
                                         bias=pabe[:, :])
                    for pk in range(2) if csub >= 3 else []:
                        zps = dupsC.tile([128, 2 * NP2], f32, tag="dps")
                        for q in range(2):
                            g = 2 * pk + q
                            nc.tensor.matmul(out=zps[64:128, q * NP2:(q + 1) * NP2],
                                             lhsT=SB['pa2rep'][32 * g:32 * g + CR, :],
                                             rhs=pat[32 * g:32 * g + CR, :],
                                             start=True, stop=True, tile_position=(32 * g, 64))
                            nc.tensor.matmul(out=zps[0:64, q * NP2:(q + 1) * NP2],
                                             lhsT=SB['idpair'][:, :],
                                             rhs=xr128(rh + 4 * pk + 2 * q, 2),
                                             start=True, stop=True, tile_position=(0, 0))
                        rr = rh + 4 * pk
                        if csub < 4:
                            continue
                        zsb = swork.tile([128, 2 * NP2], bf16, tag="zsb")
                        nc.scalar.activation(out=zsb[64:128, 0:2 * NP2], in_=zps[64:128, 0:2 * NP2],
                                             func=AF.Sigmoid, bias=pa2bt[64:128, :])
                        sc_sb = swork.tile([128, 2 * NP2], bf16, tag="s_sb")
                        nc.vector.tensor_scalar(out=sc_sb[64:128, 0:2 * NP2],
                                                in0=xrap(1, rr, 4),
                                                scalar1=ca128[64:128, 0:1], scalar2=None,
                                                op0=OP.mult)
                        nc.vector.scalar_tensor_tensor(
                            out=xrap(1, rr, 4), in0=sc_sb[64:128, 0:2 * NP2],
                            scalar=cab[64:128, 0:1], in1=zsb[64:128, 0:2 * NP2],
                            op0=OP.add, op1=OP.mult,
                            accum_out=st_r3b[64:128, scol3[0]:scol3[0] + 1])
                        scol3[0] += 1
            r3_sum = sst.tile([64, 1], f32, tag="r3_sum")
            if scol3[0]:
                nc.scalar.activation(out=st_r3[:, 0:scol3[0]], in_=st_r3b[64:128, 0:scol3[0]],
                                     func=AF.Copy)
                nc.vector.tensor_reduce(out=r3_sum[:, :], in_=st_r3[:, 0:scol3[0]], axis=X, op=OP.add)
            else:
                nc.vector.memset(r3_sum[:, :], 0.0)

        if phases.startswith('ABC') and phases != 'ABCD':
            dbg_dump_res()
        # ---- AllReduce #3: [4, 64] res3 sums
        ar3sb = sst.tile([64, 4], f32, tag="ar3sb")
        for bb in range(4):
            nc.vector.tensor_tensor(out=ar3sb[:, bb:bb + 1], in0=r3_sum[:, :],
                                    in1=myb[:, bb:bb + 1], op=OP.mult)
        ar3i = dram.tile([4, 64], f32, tag="ar3i")
        ar3o = dram.tile([4, 64], f32, tag="ar3o")
        for k in range(4):
            nc.sync.dma_start(out=ar3i[k:k + 1, :], in_=ar3sb[:, k:k + 1])
        if use_cc:
            nc.gpsimd.collective_compute("AllReduce", OP.add, replica_groups=[list(range(NCORES))],
                                         ins=[ar3i.opt()], outs=[ar3o.opt()])
        else:
            nc.sync.dma_start(out=ar3o[:, :], in_=ar3i[:, :])
        arg3 = sst.tile([64, 4], f32, tag="arg3")
        for k in range(4):
            nc.sync.dma_start(out=arg3[:, k:k + 1], in_=ar3o[k:k + 1, :])

        # ================= PHASE D : tdp2 =================
        if phases == 'ABCD':
            t2psD, dupsD = t2psP, dpsP
            r3my = sst.tile([64, 1], f32, tag="r3my")
            sel_my(r3my[:, :], arg3, [0, 1, 2, 3])
            t2ws2, Dp2 = tdp_scalar_math(2, dupsD, r3my[:, :], xs_my[:, :],
                                         arg3[:, 0:1], arg1[:, 1:2],
                                         sc['twf2'], sc['twf2b'])
            for b_ in range(nbands):
                r0 = 16 * b_

                def res4_cb(pair, rfirst, nprows, dps, _r0=r0):
                    ost = swork.tile([128, 4 * W], f32, tag="outst")
                    nc.vector.tensor_tensor(out=ost[0:64, 0:nprows * W],
                                            in0=xrap(0, rfirst, nprows),
                                            in1=dps[0:64, 0:nprows * W], op=OP.add)
                    nc.sync.dma_start(
                        out=OUT.ap()[:, rfirst * W:(rfirst + nprows) * W],
                        in_=ost[0:64, 0:nprows * W])

                tdp_band(2, r0, 16, t2ws2, Dp2, t2psD, dupsD, res4_cb)

    nc.compile()
    return nc


def make_inputs(x_np, params, H_sh, W, Hfull_arg=None):
    """Build in_maps for the 8 cores. x_np: [4, 64, Hfull, W] f32."""
    Hfull = x_np.shape[2]
    Wp = W + 2
    L = (H_sh + 4) * Wp
    cst = prep_params(params)
    in_maps = []
    for c in range(NCORES):
        b, half = c // 2, c % 2
        r0 = half * H_sh
        slab = np.zeros((64, H_sh + 4, Wp), np.float32)
        lo, hi = r0 - 2, r0 + H_sh + 2
        slo, shi = max(lo, 0), min(hi, Hfull)
        slab[:, slo - lo:shi - lo, 1:W + 1] = x_np[b, :, slo:shi, :]
        m = {n: cst[n] for n in cst}
        m['xs'] = slab.reshape(64, L).astype(bf)
        oh = np.zeros((64, 4), np.float32)
        oh[:, b] = 1.0
        m['myb'] = oh
        em = np.ones((64, 2), np.float32)
        if half == 0:
            em[:, 0] = 0.0
        if half == (Hfull // H_sh) - 1:
            em[:, 1] = 0.0
        m['edgemask'] = em
        in_maps.append(m)
    return in_maps


def run(x_np, params, H_sh=128, W=256, nc_cached=None, use_cc=True):
    p = params
    twf1 = _f32(p['twf1_w'])
    twf2 = _f32(p['twf2_w'])
    sc = {'twf1': twf1.tolist(), 'twf1b': _f32(p['twf1_b']).tolist(),
          'twf2': twf2.tolist(), 'twf2b': _f32(p['twf2_b']).tolist(),
          'pa2b': float(_f32(p['pa_b2']).reshape(())),
          'inv_hw': 1.0 / (x_np.shape[2] * W), 'inv_c': 1.0 / 64.0}
    nc = nc_cached or build_program(H_sh, W, sc, use_cc=use_cc)
    in_maps = make_inputs(x_np, params, H_sh, W)
    res = run_bass_kernel_spmd(nc, in_maps, list(range(NCORES)), trace=trace)
    run.last_exec_ns = getattr(res, 'exec_time_ns', None)
    Hfull = x_np.shape[2]
    out = np.zeros((4, 64, Hfull, W), np.float32)
    for c in range(NCORES):
        b, half = c // 2, c % 2
        out[b, :, half * H_sh:(half + 1) * H_sh, :] = \
            res.results[c]["out"].reshape(64, H_sh, W)
    return out, nc


def kernel(x, params):
    x_np = np.asarray(x, np.float32)
    out, _ = run(x_np, params, H_sh=128, W=256)
    return out
